# revision 22
# baseline (speedup 1.0000x reference)
"""MinLSTM cell (Heinsen-scan reference) as a Bass/Tile kernel for 8 trn2 NeuronCores.

Linear-space rewrite of the reference's log-space scan:
    h_t = f'_t h_{t-1} + (1 - f'_t) g(pre_h_t),   h_0 = 1e-6
with f' = sigmoid(pre_f+b_f) / (sigmoid(pre_f+b_f) + sigmoid(pre_i+b_i)) and
g(x) = x>=0 ? x+0.5 : sigmoid(x) = max(x+0.5, sigmoid(x)).

Distribution: data-parallel over batch N=8, one batch element per core, W/b
replicated. Device layout: channels on SBUF partitions (4 c-tiles of 128),
sequence along the free dim.

Per-core engine assignment (chunks of <=2048 along L):
  PE : F/I gate matmuls in fp8 E4M3 (x*16, W*64) with DoubleRow perf mode
       (2 k-tiles of 128 contracted per pass), H gate matmul in fp16.
  ACT: sf = sigmoid(psF/1024 + b_f); si = sigmoid(psI/1024 + b_i);
       sh = sigmoid(psH + b_h); on ACT-chunks also rl = psH + b_h + 0.5.
  DVE: fp = FRACT_FAST_ANT(sf, si) = sf/(sf+si), a custom fused op:
       bit-trick reciprocal seed + one Newton step + multiply (~0.17% max
       rel err), replacing the SWDGE add + fp32 recip + multiply chain.
       On DVE-chunks htil = HTIL_FUSED_ANT(psH, sh) = max(psH+b_h05, sh).
       h = tensor_tensor_scan(d0=fp, d1=wv, mult, subtract).
  GPS: fpm1 = fp - 1 (tensor_scalar); SWDGE accumulate-max (sh into rl ->
       htil, ACT-chunks) and accumulate-mult (htil into fpm1 -> wv).
  SP : all HBM loads/stores.
"""

import os
import sys

import numpy as np

sys.path.insert(0, "/opt/trn_rl_repo")

import ml_dtypes  # noqa: E402

import concourse.bass as bass  # noqa: E402
import concourse.tile as tile  # noqa: E402
from concourse import bacc, mybir  # noqa: E402
from concourse import dve_ops  # noqa: E402
from concourse.dve_spec import (  # noqa: E402
    AluOp,
    Bin,
    C0,
    C1,
    Spec,
    lower,
    maxx,
)
from concourse.dve_uop import DveOpSpec  # noqa: E402

N, L, H_IN, H = 8, 4096, 512, 512
H3 = 3 * H
P = 128
NK = H_IN // P  # 4 k-blocks of the contraction dim
NCT = H // P  # 4 channel tiles
LT = 512  # psum/matmul l-tile (one PSUM bank of fp32)
LH = 2048  # l-granularity of the big elementwise ops
F32 = mybir.dt.float32
F16 = mybir.dt.float16
F8 = mybir.dt.float8e4
Alu = mybir.AluOpType
Act = mybir.ActivationFunctionType
DR = mybir.MatmulPerfMode.DoubleRow

HX_INIT = 1e-6
XS, WS = 16.0, 64.0  # fp8 scale for x and W (TRN E4M3 max is +-240)
FR_C0, FR_C1 = -0.23549792, 2.0017324  # recip bit-seed Chebyshev consts

_cached_nc = {}
_fract_op = None
_htil_op = None


def _register_dve_ops():
    """Author + register the two fused custom DVE ops (process-local registry,
    compiled into the NEFF's per-kernel DVE table)."""
    global _fract_op, _htil_op
    if _fract_op is not None:
        return _fract_op, _htil_op

    def _np_recip_seed_nr1(s, c0, c1):
        ns = (~s.view(np.int32)).view(np.float32)
        y0 = ns * c0
        return y0 * (c1 - s * y0)

    def _ref_fract(in0, in1, c0, c1, c2):
        sf = in0.astype(np.float32)
        s = sf + in1.astype(np.float32)
        return sf * _np_recip_seed_nr1(s, c0, c1)

    _s = Src0 = None  # placeholder to appease linters
    from concourse.dve_spec import Src0, Src1  # noqa: E402

    s_expr = Src0 + Src1
    not_s = Bin(AluOp.BITWISE_NOT, s_expr, s_expr)
    y0 = not_s * C0
    y1 = y0 * (C1 - s_expr * y0)
    fract_spec = Spec(body=Src0 * y1, reference=_ref_fract)

    def _ref_htil(in0, in1, c0, c1, c2):
        return np.maximum(in0.astype(np.float32) + c0, in1.astype(np.float32))

    htil_spec = Spec(body=maxx(Src0 + C0, Src1), reference=_ref_htil)

    ops = []
    for name, spec in (
        ("FRACT_FAST_ANT", fract_spec),
        ("HTIL_FUSED_ANT", htil_spec),
    ):
        existing = next((o for o in dve_ops.OPS if o.name == name), None)
        if existing is not None:
            ops.append(existing)
            continue
        row = dve_ops._CUSTOM_DVE_ROW_BASE + len(dve_ops.OPS)
        shas = {}
        for ver in ("v3",):
            tmp = DveOpSpec(
                name=name,
                opcode=row,
                uops=lower(spec, ver=ver),
                rd1_en=True,
            )
            shas[ver] = tmp.sha(ver)
        op = dve_ops.DveOp(name=name, spec=spec, subdim=False, uops_sha=shas)
        dve_ops.OPS.append(op)
        dve_ops._SUB_OPCODE_FOR_NAME[name] = row
        dve_ops.CUSTOM_DVE_SPECS[name] = spec
        ops.append(op)
    _fract_op, _htil_op = ops
    return _fract_op, _htil_op


def build_program(L=L, LH=LH):
    key = (L, LH)
    if key in _cached_nc:
        return _cached_nc[key]
    fract_op, htil_op = _register_dve_ops()

    nc = bacc.Bacc()
    xT16_d = nc.dram_tensor("xT16", [H_IN, L], F16, kind="ExternalInput")
    xT8_d = nc.dram_tensor("xT8", [H_IN, L], F8, kind="ExternalInput")
    w16_d = nc.dram_tensor("w16", [H_IN, NCT * P], F16, kind="ExternalInput")
    w8_d = nc.dram_tensor("w8", [H_IN, NCT * 2 * P], F8, kind="ExternalInput")
    bias_d = nc.dram_tensor("bias", [P, 16], F32, kind="ExternalInput")
    out_d = nc.dram_tensor("out", [H, L], F16, kind="ExternalOutput")

    CW8 = 2 * P  # fp8 weight cols per c-tile: [F_c | I_c]

    with tile.TileContext(nc) as tc:
        with (
            tc.tile_pool(name="const", bufs=1) as const_pool,
            tc.tile_pool(name="gates", bufs=2) as gates_pool,
            tc.tile_pool(name="tail", bufs=4) as tail_pool,
            tc.tile_pool(name="scanbuf", bufs=2) as scan_pool,
            tc.tile_pool(name="psum", bufs=2, space="PSUM") as psum_pool,
        ):
            # Warmup activation: absorbs the one-time sigmoid act-table load.
            warm = const_pool.tile([P, 8], F32)
            nc.vector.memset(warm[:], 0.0)
            nc.scalar.activation(warm[:], warm[:], Act.Sigmoid)
            # PE warmup: garbage matmuls with no deps so the HAM clock gate
            # reaches 2.4GHz while the first DMAs are in flight.
            wup = const_pool.tile([P, P], F16)
            nc.vector.memset(wup[:], 0.0)
            wup_ps = psum_pool.tile([P, P], F32, tag="ps")
            for _ in range(16):
                nc.tensor.matmul(wup_ps[:], wup[:], wup[:], start=True, stop=True)

            xT16_sb = const_pool.tile([P, NK, L], F16)
            xT8_sb = const_pool.tile([P, NK, L], F8)
            w16_sb = const_pool.tile([P, NK, NCT * P], F16)
            w8_sb = const_pool.tile([P, NK, NCT * CW8], F8)
            bias_sb = const_pool.tile([P, 16], F32)

            # Load order prioritizes chunk-0's critical path (warmup is short,
            # so the first real matmuls need w8-c0 + x8[0:512] ASAP), then
            # streams the remaining weights and x chunks.
            w8_r = w8_d.rearrange("(ki p) o -> p ki o", p=P)
            w16_r = w16_d.rearrange("(ki p) o -> p ki o", p=P)
            xT16_r = xT16_d.rearrange("(ki p) l -> p ki l", p=P)
            xT8_r = xT8_d.rearrange("(ki p) l -> p ki l", p=P)
            if L >= 4096:
                xchunks = [512, 512, 1024] + [2048] * ((L - 2048) // 2048)
            else:
                xchunks = [512] * (L // 512)

            nc.sync.dma_start(w8_sb[:, :, 0:CW8], w8_r[:, :, 0:CW8])
            nc.sync.dma_start(
                xT8_sb[:, :, 0 : xchunks[0]], xT8_r[:, :, 0 : xchunks[0]]
            )
            nc.sync.dma_start(w16_sb[:, :, 0:P], w16_r[:, :, 0:P])
            nc.sync.dma_start(bias_sb[:], bias_d[:])
            nc.sync.dma_start(
                xT16_sb[:, :, 0 : xchunks[0]], xT16_r[:, :, 0 : xchunks[0]]
            )
            for cg in range(1, NCT):
                nc.sync.dma_start(
                    w8_sb[:, :, cg * CW8 : (cg + 1) * CW8],
                    w8_r[:, :, cg * CW8 : (cg + 1) * CW8],
                )
                nc.sync.dma_start(
                    w16_sb[:, :, cg * P : (cg + 1) * P],
                    w16_r[:, :, cg * P : (cg + 1) * P],
                )
            xoff = xchunks[0]
            for xch in xchunks[1:]:
                nc.sync.dma_start(
                    xT8_sb[:, :, xoff : xoff + xch],
                    xT8_r[:, :, xoff : xoff + xch],
                )
                nc.sync.dma_start(
                    xT16_sb[:, :, xoff : xoff + xch],
                    xT16_r[:, :, xoff : xoff + xch],
                )
                xoff += xch

            # Near-c-major emission with one swap: c1's small first chunk is
            # emitted before c0's last chunk (hides the c0->c1 seam on PE).
            if L >= 4096:
                big = (L - 4096) // 2048
                clists = {
                    0: [512, 512, 1024] + [2048] * (big + 1),
                    1: [512, 1536] + [2048] * (big + 1),
                    2: [2048] * (big + 2),
                    3: [2048] * (big + 1) + [1536, 512],
                }
                # Near-c-major with one hoist (c1's small first chunk before
                # c0's last): hides the c0->c1 seam. c3 trails small chunks so
                # the exposed end-of-kernel tail chain is short.
                order = [(0, 0), (0, 1), (0, 2), (1, 0), (0, 3)]
                order += [(1, j) for j in range(1, len(clists[1]))]
                order += [(2, j) for j in range(len(clists[2]))]
                order += [(3, j) for j in range(len(clists[3]))]
            else:
                clists = {c: [512] * (L // 512) for c in range(NCT)}
                order = [(c, j) for c in range(NCT)
                         for j in range(len(clists[c]))]

            hvs = {}
            lsoff = {c: 0 for c in range(NCT)}
            # Software pipelining: each chunk's (fpm1, wv, scan, store) tail is
            # emitted AFTER the next chunk's head so the SWDGE htil-add latency
            # never bubbles the in-order DVE queue.
            pending = []

            def emit_tail(tail):
                c, ls, LHC, fp, htl, hv = tail
                fpm1 = gates_pool.tile([P, LHC], F16, tag="fpm1")
                nc.vector.tensor_scalar_add(fpm1[:], fp[:], -1.0)
                wv = gates_pool.tile([P, LHC], F16, tag="wv")
                nc.vector.tensor_tensor(wv[:], fpm1[:], htl[:], Alu.mult)
                init = HX_INIT if ls == 0 else hv[:, ls - 1 : ls]
                nc.vector.tensor_tensor_scan(
                    hv[:, ls : ls + LHC], fp[:], wv[:], init,
                    Alu.mult, Alu.subtract,
                )
                nc.sync.dma_start(
                    out_d[c * P : (c + 1) * P, ls : ls + LHC],
                    hv[:, ls : ls + LHC],
                )

            for ei, (c, lh) in enumerate(order):
                if lh == 0:
                    hvs[c] = scan_pool.tile([P, L], F16, tag="hv", name=f"hv{c}")
                hv = hvs[c]
                LHC = clists[c][lh]
                ls = lsoff[c]

                sf = gates_pool.tile([P, LHC], F16, tag="sf")
                si = gates_pool.tile([P, LHC], F16, tag="si")
                fp = tail_pool.tile([P, LHC], F16, tag="fp")
                sh = gates_pool.tile([P, LHC], F16, tag="sh")
                htl = tail_pool.tile([P, LHC], F16, tag="htl")

                def mms8(ps, ocol):
                    # fp8 DoubleRow: 2 k-tiles of 128 contracted per pass
                    for j in range(LHC // LT):
                        xk = slice(ls + j * LT, ls + (j + 1) * LT)
                        jl = slice(j * LT, (j + 1) * LT)
                        for kp in range(NK // 2):
                            nc.tensor.matmul(
                                ps[:, jl],
                                w8_sb[:, 2 * kp : 2 * kp + 2, ocol : ocol + P],
                                xT8_sb[:, 2 * kp : 2 * kp + 2, xk],
                                start=kp == 0,
                                stop=kp == NK // 2 - 1,
                                perf_mode=DR,
                            )

                def mms16(ps, ocol):
                    for j in range(LHC // LT):
                        xk = slice(ls + j * LT, ls + (j + 1) * LT)
                        jl = slice(j * LT, (j + 1) * LT)
                        for ki in range(NK):
                            nc.tensor.matmul(
                                ps[:, jl],
                                w16_sb[:, ki, ocol : ocol + P],
                                xT16_sb[:, ki, xk],
                                start=ki == 0,
                                stop=ki == NK - 1,
                            )

                # F gate
                psF = psum_pool.tile([P, LHC], F32, tag="ps")
                mms8(psF, c * CW8)
                nc.scalar.activation(
                    sf[:], psF[:], Act.Sigmoid,
                    bias=bias_sb[:, 0 * NCT + c : 0 * NCT + c + 1],
                    scale=1.0 / (XS * WS),
                )
                # I gate
                psI = psum_pool.tile([P, LHC], F32, tag="ps")
                mms8(psI, c * CW8 + P)
                nc.scalar.activation(
                    si[:], psI[:], Act.Sigmoid,
                    bias=bias_sb[:, 1 * NCT + c : 1 * NCT + c + 1],
                    scale=1.0 / (XS * WS),
                )
                # f' = sf/(sf+si), one fused DVE op
                nc.vector._custom_dve(
                    fract_op, out=fp[:], in0=sf[:], in1=si[:],
                    s0=FR_C0, s1=FR_C1,
                )

                # H gate
                psH = psum_pool.tile([P, LHC], F32, tag="ps")
                mms16(psH, c * P)
                nc.scalar.activation(
                    sh[:], psH[:], Act.Sigmoid,
                    bias=bias_sb[:, 2 * NCT + c : 2 * NCT + c + 1],
                )
                # htil = relu(psH + b_h) + min(sh, 0.5), exact identity for g:
                # relu on ACT, min on DVE (4x TS), add via SWDGE (off-engine).
                mn = gates_pool.tile([P, LHC], F16, tag="mn")
                nc.scalar.activation(
                    htl[:], psH[:], Act.Relu,
                    bias=bias_sb[:, 2 * NCT + c : 2 * NCT + c + 1],
                )
                nc.vector.tensor_scalar_min(mn[:], sh[:], 0.5)
                if ei == len(order) - 1:
                    # Final chunk: SWDGE round-trip would sit exposed at the
                    # end of the kernel; do the htil add on DVE instead.
                    htl2 = tail_pool.tile([P, LHC], F16, tag="htl2")
                    nc.vector.tensor_tensor(htl2[:], htl[:], mn[:], Alu.add)
                    htl = htl2
                else:
                    nc.gpsimd.dma_start(
                        out=htl[:], in_=mn[:], accum_op=Alu.add
                    )
                pending.append((c, ls, LHC, fp, htl, hv))
                if len(pending) > 2:
                    emit_tail(pending.pop(0))
                lsoff[c] += LHC
            while pending:
                emit_tail(pending.pop(0))

    nc.compile()
    _cached_nc[key] = nc
    return nc


def reorder_w8(W: np.ndarray) -> np.ndarray:
    """fp8 weights, [H_IN, NCT*2P] with per-c-tile [F_c | I_c] col groups."""
    Wf, Wi = W[:H], W[H : 2 * H]
    cols = []
    for c in range(NCT):
        cols.append(Wf[c * P : (c + 1) * P])
        cols.append(Wi[c * P : (c + 1) * P])
    w = np.concatenate(cols, axis=0)  # [NCT*2P, H_IN]
    w8 = np.clip(w.T * WS, -240.0, 240.0).astype(ml_dtypes.float8_e4m3fn)
    return np.ascontiguousarray(w8)


def reorder_w16(W: np.ndarray) -> np.ndarray:
    Wh = W[2 * H :]
    return np.ascontiguousarray(Wh.T.astype(np.float16))


def make_bias(b: np.ndarray) -> np.ndarray:
    b32 = np.asarray(b, dtype=np.float32)
    bias = np.empty((P, 16), dtype=np.float32)
    for c in range(NCT):
        bias[:, 0 * NCT + c] = b32[0 * H + c * P : 0 * H + (c + 1) * P]
        bias[:, 1 * NCT + c] = b32[1 * H + c * P : 1 * H + (c + 1) * P]
        bias[:, 2 * NCT + c] = b32[2 * H + c * P : 2 * H + (c + 1) * P]
        bias[:, 3 * NCT + c] = b32[2 * H + c * P : 2 * H + (c + 1) * P] + 0.5
    return bias


def _prep_core_inputs(x_n, w16, w8, bias):
    xT = np.ascontiguousarray(x_n.T)
    return {
        "xT16": xT.astype(np.float16),
        "xT8": np.clip(xT * XS, -240.0, 240.0).astype(ml_dtypes.float8_e4m3fn),
        "w16": w16,
        "w8": w8,
        "bias": bias,
    }


def kernel(x: np.ndarray, W: np.ndarray, b: np.ndarray) -> np.ndarray:
    from concourse.bass_utils import run_bass_kernel_spmd

    nc = build_program()

    W = np.asarray(W)
    w16 = reorder_w16(W)
    w8 = reorder_w8(W)
    bias = make_bias(b)

    in_maps = [_prep_core_inputs(np.asarray(x[n]), w16, w8, bias) for n in range(N)]
    res = run_bass_kernel_spmd(nc, in_maps, list(range(N)))

    out = np.empty((N, L, H), dtype=np.float32)
    for n in range(N):
        out[n] = res.results[n]["out"].T.astype(np.float32)
    return out


# revision 24
# speedup vs baseline: 1.0309x; 1.0309x over previous
"""MinLSTM cell (Heinsen-scan reference) as a Bass/Tile kernel for 8 trn2 NeuronCores.

Linear-space rewrite of the reference's log-space scan:
    h_t = f'_t h_{t-1} + (1 - f'_t) g(pre_h_t),   h_0 = 1e-6
with f' = sigmoid(pre_f+b_f) / (sigmoid(pre_f+b_f) + sigmoid(pre_i+b_i)) and
g(x) = x>=0 ? x+0.5 : sigmoid(x) = max(x+0.5, sigmoid(x)).

Distribution: data-parallel over batch N=8, one batch element per core, W/b
replicated. Device layout: channels on SBUF partitions (4 c-tiles of 128),
sequence along the free dim.

Per-core engine assignment (chunks of <=2048 along L):
  PE : F/I gate matmuls in fp8 E4M3 (x*16, W*64) with DoubleRow perf mode
       (2 k-tiles of 128 contracted per pass), H gate matmul in fp16.
  ACT: sf = sigmoid(psF/1024 + b_f); si = sigmoid(psI/1024 + b_i);
       sh = sigmoid(psH + b_h); on ACT-chunks also rl = psH + b_h + 0.5.
  DVE: fp = FRACT_FAST_ANT(sf, si) = sf/(sf+si), a custom fused op:
       bit-trick reciprocal seed + one Newton step + multiply (~0.17% max
       rel err), replacing the SWDGE add + fp32 recip + multiply chain.
       On DVE-chunks htil = HTIL_FUSED_ANT(psH, sh) = max(psH+b_h05, sh).
       h = tensor_tensor_scan(d0=fp, d1=wv, mult, subtract).
  GPS: fpm1 = fp - 1 (tensor_scalar); SWDGE accumulate-max (sh into rl ->
       htil, ACT-chunks) and accumulate-mult (htil into fpm1 -> wv).
  SP : all HBM loads/stores.
"""

import os
import sys

import numpy as np

sys.path.insert(0, "/opt/trn_rl_repo")

import ml_dtypes  # noqa: E402

import concourse.bass as bass  # noqa: E402
import concourse.tile as tile  # noqa: E402
from concourse import bacc, mybir  # noqa: E402
from concourse import dve_ops  # noqa: E402
from concourse.dve_spec import (  # noqa: E402
    AluOp,
    Bin,
    C0,
    C1,
    Spec,
    lower,
    maxx,
)
from concourse.dve_uop import DveOpSpec  # noqa: E402

N, L, H_IN, H = 8, 4096, 512, 512
H3 = 3 * H
P = 128
NK = H_IN // P  # 4 k-blocks of the contraction dim
NCT = H // P  # 4 channel tiles
LT = 512  # psum/matmul l-tile (one PSUM bank of fp32)
LH = 2048  # l-granularity of the big elementwise ops
F32 = mybir.dt.float32
F16 = mybir.dt.float16
F8 = mybir.dt.float8e4
Alu = mybir.AluOpType
Act = mybir.ActivationFunctionType
DR = mybir.MatmulPerfMode.DoubleRow

HX_INIT = 1e-6
XS, WS = 16.0, 64.0  # fp8 scale for x and W (TRN E4M3 max is +-240)
FR_C0, FR_C1 = -0.23549792, 2.0017324  # recip bit-seed Chebyshev consts

_cached_nc = {}
_fract_op = None
_htil_op = None


def _register_dve_ops():
    """Author + register the two fused custom DVE ops (process-local registry,
    compiled into the NEFF's per-kernel DVE table)."""
    global _fract_op, _htil_op
    if _fract_op is not None:
        return _fract_op, _htil_op

    def _np_recip_seed_nr1(s, c0, c1):
        ns = (~s.view(np.int32)).view(np.float32)
        y0 = ns * c0
        return y0 * (c1 - s * y0)

    def _ref_fract(in0, in1, c0, c1, c2):
        sf = in0.astype(np.float32)
        s = sf + in1.astype(np.float32)
        return sf * _np_recip_seed_nr1(s, c0, c1)

    _s = Src0 = None  # placeholder to appease linters
    from concourse.dve_spec import Src0, Src1  # noqa: E402

    s_expr = Src0 + Src1
    not_s = Bin(AluOp.BITWISE_NOT, s_expr, s_expr)
    y0 = not_s * C0
    y1 = y0 * (C1 - s_expr * y0)
    fract_spec = Spec(body=Src0 * y1, reference=_ref_fract)

    def _ref_htil(in0, in1, c0, c1, c2):
        return np.maximum(in0.astype(np.float32) + c0, in1.astype(np.float32))

    htil_spec = Spec(body=maxx(Src0 + C0, Src1), reference=_ref_htil)

    ops = []
    for name, spec in (
        ("FRACT_FAST_ANT", fract_spec),
        ("HTIL_FUSED_ANT", htil_spec),
    ):
        existing = next((o for o in dve_ops.OPS if o.name == name), None)
        if existing is not None:
            ops.append(existing)
            continue
        row = dve_ops._CUSTOM_DVE_ROW_BASE + len(dve_ops.OPS)
        shas = {}
        for ver in ("v3",):
            tmp = DveOpSpec(
                name=name,
                opcode=row,
                uops=lower(spec, ver=ver),
                rd1_en=True,
            )
            shas[ver] = tmp.sha(ver)
        op = dve_ops.DveOp(name=name, spec=spec, subdim=False, uops_sha=shas)
        dve_ops.OPS.append(op)
        dve_ops._SUB_OPCODE_FOR_NAME[name] = row
        dve_ops.CUSTOM_DVE_SPECS[name] = spec
        ops.append(op)
    _fract_op, _htil_op = ops
    return _fract_op, _htil_op


def build_program(L=L, LH=LH):
    key = (L, LH)
    if key in _cached_nc:
        return _cached_nc[key]
    fract_op, htil_op = _register_dve_ops()

    nc = bacc.Bacc()
    xT16_d = nc.dram_tensor("xT16", [H_IN, L], F16, kind="ExternalInput")
    xT8_d = nc.dram_tensor("xT8", [H_IN, L], F8, kind="ExternalInput")
    w16_d = nc.dram_tensor("w16", [H_IN, NCT * P], F16, kind="ExternalInput")
    w8_d = nc.dram_tensor("w8", [H_IN, NCT * 2 * P], F8, kind="ExternalInput")
    bias_d = nc.dram_tensor("bias", [P, 16], F32, kind="ExternalInput")
    out_d = nc.dram_tensor("out", [H, L], F16, kind="ExternalOutput")

    CW8 = 2 * P  # fp8 weight cols per c-tile: [F_c | I_c]

    with tile.TileContext(nc) as tc:
        with (
            tc.tile_pool(name="const", bufs=1) as const_pool,
            tc.tile_pool(name="gates", bufs=3) as gates_pool,
            tc.tile_pool(name="tail", bufs=3) as tail_pool,
            tc.tile_pool(name="scanbuf", bufs=2) as scan_pool,
            tc.tile_pool(name="psum", bufs=2, space="PSUM") as psum_pool,
        ):
            # Warmup activation: absorbs the one-time sigmoid act-table load.
            warm = const_pool.tile([P, 8], F32)
            nc.vector.memset(warm[:], 0.0)
            nc.scalar.activation(warm[:], warm[:], Act.Sigmoid)
            # PE warmup: garbage matmuls with no deps so the HAM clock gate
            # reaches 2.4GHz while the first DMAs are in flight.
            wup = const_pool.tile([P, P], F16)
            nc.vector.memset(wup[:], 0.0)
            wup_ps = psum_pool.tile([P, P], F32, tag="ps")
            for _ in range(16):
                nc.tensor.matmul(wup_ps[:], wup[:], wup[:], start=True, stop=True)

            xT16_sb = const_pool.tile([P, NK, L], F16)
            xT8_sb = const_pool.tile([P, NK, L], F8)
            w16_sb = const_pool.tile([P, NK, NCT * P], F16)
            w8_sb = const_pool.tile([P, NK, NCT * CW8], F8)
            bias_sb = const_pool.tile([P, 16], F32)

            # Load order prioritizes chunk-0's critical path (warmup is short,
            # so the first real matmuls need w8-c0 + x8[0:512] ASAP), then
            # streams the remaining weights and x chunks.
            w8_r = w8_d.rearrange("(ki p) o -> p ki o", p=P)
            w16_r = w16_d.rearrange("(ki p) o -> p ki o", p=P)
            xT16_r = xT16_d.rearrange("(ki p) l -> p ki l", p=P)
            xT8_r = xT8_d.rearrange("(ki p) l -> p ki l", p=P)
            if L >= 4096:
                xchunks = [512, 512, 1024] + [2048] * ((L - 2048) // 2048)
            else:
                xchunks = [512] * (L // 512)

            nc.sync.dma_start(w8_sb[:, :, 0:CW8], w8_r[:, :, 0:CW8])
            nc.sync.dma_start(
                xT8_sb[:, :, 0 : xchunks[0]], xT8_r[:, :, 0 : xchunks[0]]
            )
            nc.sync.dma_start(w16_sb[:, :, 0:P], w16_r[:, :, 0:P])
            nc.sync.dma_start(bias_sb[:], bias_d[:])
            nc.sync.dma_start(
                xT16_sb[:, :, 0 : xchunks[0]], xT16_r[:, :, 0 : xchunks[0]]
            )
            for cg in range(1, NCT):
                nc.sync.dma_start(
                    w8_sb[:, :, cg * CW8 : (cg + 1) * CW8],
                    w8_r[:, :, cg * CW8 : (cg + 1) * CW8],
                )
                nc.sync.dma_start(
                    w16_sb[:, :, cg * P : (cg + 1) * P],
                    w16_r[:, :, cg * P : (cg + 1) * P],
                )
            xoff = xchunks[0]
            for xch in xchunks[1:]:
                nc.sync.dma_start(
                    xT8_sb[:, :, xoff : xoff + xch],
                    xT8_r[:, :, xoff : xoff + xch],
                )
                nc.sync.dma_start(
                    xT16_sb[:, :, xoff : xoff + xch],
                    xT16_r[:, :, xoff : xoff + xch],
                )
                xoff += xch

            # Near-c-major emission with one swap: c1's small first chunk is
            # emitted before c0's last chunk (hides the c0->c1 seam on PE).
            if L >= 4096:
                big = (L - 4096) // 2048
                clists = {
                    0: [512, 512, 1024] + [2048] * (big + 1),
                    1: [512, 1536] + [2048] * (big + 1),
                    2: [2048] * (big + 2),
                    3: [2048] * (big + 1) + [1536, 512],
                }
                # Near-c-major with one hoist (c1's small first chunk before
                # c0's last): hides the c0->c1 seam. c3 trails small chunks so
                # the exposed end-of-kernel tail chain is short.
                order = [(0, 0), (0, 1), (0, 2), (1, 0), (0, 3)]
                order += [(1, j) for j in range(1, len(clists[1]))]
                order += [(2, j) for j in range(len(clists[2]))]
                order += [(3, j) for j in range(len(clists[3]))]
            else:
                clists = {c: [512] * (L // 512) for c in range(NCT)}
                order = [(c, j) for c in range(NCT)
                         for j in range(len(clists[c]))]

            hvs = {}
            lsoff = {c: 0 for c in range(NCT)}
            # Software pipelining: each chunk's (fpm1, wv, scan, store) tail is
            # emitted AFTER the next chunk's head so the SWDGE htil-add latency
            # never bubbles the in-order DVE queue.
            pending = []

            def emit_tail(tail):
                c, ls, LHC, fp, fpm1, htl, hv = tail
                wv = gates_pool.tile([P, LHC], F16, tag="wv")
                nc.vector.tensor_tensor(wv[:], fpm1[:], htl[:], Alu.mult)
                init = HX_INIT if ls == 0 else hv[:, ls - 1 : ls]
                nc.vector.tensor_tensor_scan(
                    hv[:, ls : ls + LHC], fp[:], wv[:], init,
                    Alu.mult, Alu.subtract,
                )
                nc.sync.dma_start(
                    out_d[c * P : (c + 1) * P, ls : ls + LHC],
                    hv[:, ls : ls + LHC],
                )

            for ei, (c, lh) in enumerate(order):
                if lh == 0:
                    hvs[c] = scan_pool.tile([P, L], F16, tag="hv", name=f"hv{c}")
                hv = hvs[c]
                LHC = clists[c][lh]
                ls = lsoff[c]

                sf = gates_pool.tile([P, LHC], F16, tag="sf")
                si = gates_pool.tile([P, LHC], F16, tag="si")
                fp = tail_pool.tile([P, LHC], F16, tag="fp")
                sh = gates_pool.tile([P, LHC], F16, tag="sh")
                htl = tail_pool.tile([P, LHC], F16, tag="htl")

                def mms8(ps, ocol):
                    # fp8 DoubleRow: 2 k-tiles of 128 contracted per pass
                    for j in range(LHC // LT):
                        xk = slice(ls + j * LT, ls + (j + 1) * LT)
                        jl = slice(j * LT, (j + 1) * LT)
                        for kp in range(NK // 2):
                            nc.tensor.matmul(
                                ps[:, jl],
                                w8_sb[:, 2 * kp : 2 * kp + 2, ocol : ocol + P],
                                xT8_sb[:, 2 * kp : 2 * kp + 2, xk],
                                start=kp == 0,
                                stop=kp == NK // 2 - 1,
                                perf_mode=DR,
                            )

                def mms16(ps, ocol):
                    for j in range(LHC // LT):
                        xk = slice(ls + j * LT, ls + (j + 1) * LT)
                        jl = slice(j * LT, (j + 1) * LT)
                        for ki in range(NK):
                            nc.tensor.matmul(
                                ps[:, jl],
                                w16_sb[:, ki, ocol : ocol + P],
                                xT16_sb[:, ki, xk],
                                start=ki == 0,
                                stop=ki == NK - 1,
                            )

                # F gate
                psF = psum_pool.tile([P, LHC], F32, tag="ps")
                mms8(psF, c * CW8)
                nc.scalar.activation(
                    sf[:], psF[:], Act.Sigmoid,
                    bias=bias_sb[:, 0 * NCT + c : 0 * NCT + c + 1],
                    scale=1.0 / (XS * WS),
                )
                # I gate
                psI = psum_pool.tile([P, LHC], F32, tag="ps")
                mms8(psI, c * CW8 + P)
                nc.scalar.activation(
                    si[:], psI[:], Act.Sigmoid,
                    bias=bias_sb[:, 1 * NCT + c : 1 * NCT + c + 1],
                    scale=1.0 / (XS * WS),
                )
                # f' = sf/(sf+si), one fused DVE op
                nc.vector._custom_dve(
                    fract_op, out=fp[:], in0=sf[:], in1=si[:],
                    s0=FR_C0, s1=FR_C1,
                )
                # fpm1 only needs fp: compute it in the head so the deferred
                # tail chain after the SWDGE htil-add is just TT + scan.
                fpm1 = tail_pool.tile([P, LHC], F16, tag="fpm1")
                nc.vector.tensor_scalar_add(fpm1[:], fp[:], -1.0)

                # H gate
                psH = psum_pool.tile([P, LHC], F32, tag="ps")
                mms16(psH, c * P)
                nc.scalar.activation(
                    sh[:], psH[:], Act.Sigmoid,
                    bias=bias_sb[:, 2 * NCT + c : 2 * NCT + c + 1],
                )
                # htil = relu(psH + b_h) + min(sh, 0.5), exact identity for g:
                # relu on ACT, min on DVE (4x TS), add via SWDGE (off-engine).
                mn = gates_pool.tile([P, LHC], F16, tag="mn")
                nc.scalar.activation(
                    htl[:], psH[:], Act.Relu,
                    bias=bias_sb[:, 2 * NCT + c : 2 * NCT + c + 1],
                )
                nc.vector.tensor_scalar_min(mn[:], sh[:], 0.5)
                nc.gpsimd.dma_start(
                    out=htl[:], in_=mn[:], accum_op=Alu.add
                )
                pending.append((c, ls, LHC, fp, fpm1, htl, hv))
                if len(pending) > 1:
                    emit_tail(pending.pop(0))
                lsoff[c] += LHC
            while pending:
                emit_tail(pending.pop(0))

    nc.compile()
    _cached_nc[key] = nc
    return nc


def reorder_w8(W: np.ndarray) -> np.ndarray:
    """fp8 weights, [H_IN, NCT*2P] with per-c-tile [F_c | I_c] col groups."""
    Wf, Wi = W[:H], W[H : 2 * H]
    cols = []
    for c in range(NCT):
        cols.append(Wf[c * P : (c + 1) * P])
        cols.append(Wi[c * P : (c + 1) * P])
    w = np.concatenate(cols, axis=0)  # [NCT*2P, H_IN]
    w8 = np.clip(w.T * WS, -240.0, 240.0).astype(ml_dtypes.float8_e4m3fn)
    return np.ascontiguousarray(w8)


def reorder_w16(W: np.ndarray) -> np.ndarray:
    Wh = W[2 * H :]
    return np.ascontiguousarray(Wh.T.astype(np.float16))


def make_bias(b: np.ndarray) -> np.ndarray:
    b32 = np.asarray(b, dtype=np.float32)
    bias = np.empty((P, 16), dtype=np.float32)
    for c in range(NCT):
        bias[:, 0 * NCT + c] = b32[0 * H + c * P : 0 * H + (c + 1) * P]
        bias[:, 1 * NCT + c] = b32[1 * H + c * P : 1 * H + (c + 1) * P]
        bias[:, 2 * NCT + c] = b32[2 * H + c * P : 2 * H + (c + 1) * P]
        bias[:, 3 * NCT + c] = b32[2 * H + c * P : 2 * H + (c + 1) * P] + 0.5
    return bias


def _prep_core_inputs(x_n, w16, w8, bias):
    xT = np.ascontiguousarray(x_n.T)
    return {
        "xT16": xT.astype(np.float16),
        "xT8": np.clip(xT * XS, -240.0, 240.0).astype(ml_dtypes.float8_e4m3fn),
        "w16": w16,
        "w8": w8,
        "bias": bias,
    }


def kernel(x: np.ndarray, W: np.ndarray, b: np.ndarray) -> np.ndarray:
    from concourse.bass_utils import run_bass_kernel_spmd

    nc = build_program()

    W = np.asarray(W)
    w16 = reorder_w16(W)
    w8 = reorder_w8(W)
    bias = make_bias(b)

    in_maps = [_prep_core_inputs(np.asarray(x[n]), w16, w8, bias) for n in range(N)]
    res = run_bass_kernel_spmd(nc, in_maps, list(range(N)))

    out = np.empty((N, L, H), dtype=np.float32)
    for n in range(N):
        out[n] = res.results[n]["out"].T.astype(np.float32)
    return out


# revision 25
# speedup vs baseline: 1.0511x; 1.0196x over previous
"""MinLSTM cell (Heinsen-scan reference) as a Bass/Tile kernel for 8 trn2 NeuronCores.

Linear-space rewrite of the reference's log-space scan:
    h_t = f'_t h_{t-1} + (1 - f'_t) g(pre_h_t),   h_0 = 1e-6
with f' = sigmoid(pre_f+b_f) / (sigmoid(pre_f+b_f) + sigmoid(pre_i+b_i)) and
g(x) = x>=0 ? x+0.5 : sigmoid(x) = max(x+0.5, sigmoid(x)).

Distribution: data-parallel over batch N=8, one batch element per core, W/b
replicated. Device layout: channels on SBUF partitions (4 c-tiles of 128),
sequence along the free dim.

Per-core engine assignment (chunks of <=2048 along L):
  PE : F/I gate matmuls in fp8 E4M3 (x*16, W*64) with DoubleRow perf mode
       (2 k-tiles of 128 contracted per pass), H gate matmul in fp16.
  ACT: sf = sigmoid(psF/1024 + b_f); si = sigmoid(psI/1024 + b_i);
       sh = sigmoid(psH + b_h); on ACT-chunks also rl = psH + b_h + 0.5.
  DVE: fp = FRACT_FAST_ANT(sf, si) = sf/(sf+si), a custom fused op:
       bit-trick reciprocal seed + one Newton step + multiply (~0.17% max
       rel err), replacing the SWDGE add + fp32 recip + multiply chain.
       On DVE-chunks htil = HTIL_FUSED_ANT(psH, sh) = max(psH+b_h05, sh).
       h = tensor_tensor_scan(d0=fp, d1=wv, mult, subtract).
  GPS: fpm1 = fp - 1 (tensor_scalar); SWDGE accumulate-max (sh into rl ->
       htil, ACT-chunks) and accumulate-mult (htil into fpm1 -> wv).
  SP : all HBM loads/stores.
"""

import os
import sys

import numpy as np

sys.path.insert(0, "/opt/trn_rl_repo")

import ml_dtypes  # noqa: E402

import concourse.bass as bass  # noqa: E402
import concourse.tile as tile  # noqa: E402
from concourse import bacc, mybir  # noqa: E402
from concourse import dve_ops  # noqa: E402
from concourse.dve_spec import (  # noqa: E402
    AluOp,
    Bin,
    C0,
    C1,
    Spec,
    lower,
    maxx,
)
from concourse.dve_uop import DveOpSpec  # noqa: E402

N, L, H_IN, H = 8, 4096, 512, 512
H3 = 3 * H
P = 128
NK = H_IN // P  # 4 k-blocks of the contraction dim
NCT = H // P  # 4 channel tiles
LT = 512  # psum/matmul l-tile (one PSUM bank of fp32)
LH = 2048  # l-granularity of the big elementwise ops
F32 = mybir.dt.float32
F16 = mybir.dt.float16
F8 = mybir.dt.float8e4
Alu = mybir.AluOpType
Act = mybir.ActivationFunctionType
DR = mybir.MatmulPerfMode.DoubleRow

HX_INIT = 1e-6
XS, WS = 16.0, 64.0  # fp8 scale for x and W (TRN E4M3 max is +-240)
FR_C0, FR_C1 = -0.23549792, 2.0017324  # recip bit-seed Chebyshev consts

_cached_nc = {}
_fract_op = None
_htil_op = None


def _register_dve_ops():
    """Author + register the two fused custom DVE ops (process-local registry,
    compiled into the NEFF's per-kernel DVE table)."""
    global _fract_op, _htil_op
    if _fract_op is not None:
        return _fract_op, _htil_op

    def _np_recip_seed_nr1(s, c0, c1):
        ns = (~s.view(np.int32)).view(np.float32)
        y0 = ns * c0
        return y0 * (c1 - s * y0)

    def _ref_fract(in0, in1, c0, c1, c2):
        sf = in0.astype(np.float32)
        s = sf + in1.astype(np.float32)
        return sf * _np_recip_seed_nr1(s, c0, c1)

    _s = Src0 = None  # placeholder to appease linters
    from concourse.dve_spec import Src0, Src1  # noqa: E402

    s_expr = Src0 + Src1
    not_s = Bin(AluOp.BITWISE_NOT, s_expr, s_expr)
    y0 = not_s * C0
    y1 = y0 * (C1 - s_expr * y0)
    fract_spec = Spec(body=Src0 * y1, reference=_ref_fract)

    def _ref_htil(in0, in1, c0, c1, c2):
        return np.maximum(in0.astype(np.float32) + c0, in1.astype(np.float32))

    htil_spec = Spec(body=maxx(Src0 + C0, Src1), reference=_ref_htil)

    ops = []
    for name, spec in (
        ("FRACT_FAST_ANT", fract_spec),
        ("HTIL_FUSED_ANT", htil_spec),
    ):
        existing = next((o for o in dve_ops.OPS if o.name == name), None)
        if existing is not None:
            ops.append(existing)
            continue
        row = dve_ops._CUSTOM_DVE_ROW_BASE + len(dve_ops.OPS)
        shas = {}
        for ver in ("v3",):
            tmp = DveOpSpec(
                name=name,
                opcode=row,
                uops=lower(spec, ver=ver),
                rd1_en=True,
            )
            shas[ver] = tmp.sha(ver)
        op = dve_ops.DveOp(name=name, spec=spec, subdim=False, uops_sha=shas)
        dve_ops.OPS.append(op)
        dve_ops._SUB_OPCODE_FOR_NAME[name] = row
        dve_ops.CUSTOM_DVE_SPECS[name] = spec
        ops.append(op)
    _fract_op, _htil_op = ops
    return _fract_op, _htil_op


def build_program(L=L, LH=LH):
    key = (L, LH)
    if key in _cached_nc:
        return _cached_nc[key]
    fract_op, htil_op = _register_dve_ops()

    nc = bacc.Bacc()
    xT16_d = nc.dram_tensor("xT16", [H_IN, L], F16, kind="ExternalInput")
    xT8_d = nc.dram_tensor("xT8", [H_IN, L], F8, kind="ExternalInput")
    w16_d = nc.dram_tensor("w16", [H_IN, NCT * P], F16, kind="ExternalInput")
    w8_d = nc.dram_tensor("w8", [H_IN, NCT * 2 * P], F8, kind="ExternalInput")
    bias_d = nc.dram_tensor("bias", [P, 16], F32, kind="ExternalInput")
    out_d = nc.dram_tensor("out", [H, L], F16, kind="ExternalOutput")

    CW8 = 2 * P  # fp8 weight cols per c-tile: [F_c | I_c]

    with tile.TileContext(nc) as tc:
        with (
            tc.tile_pool(name="const", bufs=1) as const_pool,
            tc.tile_pool(name="gates", bufs=2) as gates_pool,
            tc.tile_pool(name="tail", bufs=3) as tail_pool,
            tc.tile_pool(name="scanbuf", bufs=2) as scan_pool,
            tc.tile_pool(name="psum", bufs=2, space="PSUM") as psum_pool,
        ):
            # Warmup activation: absorbs the one-time sigmoid act-table load.
            warm = const_pool.tile([P, 8], F32)
            nc.vector.memset(warm[:], 0.0)
            nc.scalar.activation(warm[:], warm[:], Act.Sigmoid)
            # PE warmup: garbage matmuls with no deps so the HAM clock gate
            # reaches 2.4GHz while the first DMAs are in flight.
            wup = const_pool.tile([P, P], F16)
            nc.vector.memset(wup[:], 0.0)
            wup_ps = psum_pool.tile([P, P], F32, tag="ps")
            for _ in range(16):
                nc.tensor.matmul(wup_ps[:], wup[:], wup[:], start=True, stop=True)

            xT16_sb = const_pool.tile([P, NK, L], F16)
            xT8_sb = const_pool.tile([P, NK, L], F8)
            w16_sb = const_pool.tile([P, NK, NCT * P], F16)
            w8_sb = const_pool.tile([P, NK, NCT * CW8], F8)
            bias_sb = const_pool.tile([P, 16], F32)

            # Load order prioritizes chunk-0's critical path (warmup is short,
            # so the first real matmuls need w8-c0 + x8[0:512] ASAP), then
            # streams the remaining weights and x chunks.
            w8_r = w8_d.rearrange("(ki p) o -> p ki o", p=P)
            w16_r = w16_d.rearrange("(ki p) o -> p ki o", p=P)
            xT16_r = xT16_d.rearrange("(ki p) l -> p ki l", p=P)
            xT8_r = xT8_d.rearrange("(ki p) l -> p ki l", p=P)
            if L >= 4096:
                xchunks = [512, 512, 1024] + [2048] * ((L - 2048) // 2048)
            else:
                xchunks = [512] * (L // 512)

            nc.sync.dma_start(w8_sb[:, :, 0:CW8], w8_r[:, :, 0:CW8])
            nc.sync.dma_start(
                xT8_sb[:, :, 0 : xchunks[0]], xT8_r[:, :, 0 : xchunks[0]]
            )
            nc.sync.dma_start(w16_sb[:, :, 0:P], w16_r[:, :, 0:P])
            nc.sync.dma_start(bias_sb[:], bias_d[:])
            nc.sync.dma_start(
                xT16_sb[:, :, 0 : xchunks[0]], xT16_r[:, :, 0 : xchunks[0]]
            )
            for cg in range(1, NCT):
                nc.sync.dma_start(
                    w8_sb[:, :, cg * CW8 : (cg + 1) * CW8],
                    w8_r[:, :, cg * CW8 : (cg + 1) * CW8],
                )
                nc.sync.dma_start(
                    w16_sb[:, :, cg * P : (cg + 1) * P],
                    w16_r[:, :, cg * P : (cg + 1) * P],
                )
            xoff = xchunks[0]
            for xch in xchunks[1:]:
                nc.sync.dma_start(
                    xT8_sb[:, :, xoff : xoff + xch],
                    xT8_r[:, :, xoff : xoff + xch],
                )
                nc.sync.dma_start(
                    xT16_sb[:, :, xoff : xoff + xch],
                    xT16_r[:, :, xoff : xoff + xch],
                )
                xoff += xch

            # Near-c-major emission with one swap: c1's small first chunk is
            # emitted before c0's last chunk (hides the c0->c1 seam on PE).
            if L >= 4096:
                big = (L - 4096) // 2048
                clists = {
                    0: [512, 512, 1024] + [2048] * (big + 1),
                    1: [512, 1536] + [2048] * (big + 1),
                    2: [2048] * (big + 2),
                    3: [2048] * (big + 1) + [1536, 512],
                }
                # Near-c-major with one hoist (c1's small first chunk before
                # c0's last): hides the c0->c1 seam. c3 trails small chunks so
                # the exposed end-of-kernel tail chain is short.
                order = [(0, 0), (0, 1), (0, 2), (1, 0), (0, 3)]
                order += [(1, j) for j in range(1, len(clists[1]))]
                order += [(2, j) for j in range(len(clists[2]))]
                order += [(3, j) for j in range(len(clists[3]))]
            else:
                clists = {c: [512] * (L // 512) for c in range(NCT)}
                order = [(c, j) for c in range(NCT)
                         for j in range(len(clists[c]))]

            hvs = {}
            lsoff = {c: 0 for c in range(NCT)}
            # Software pipelining: each chunk's (fpm1, wv, scan, store) tail is
            # emitted AFTER the next chunk's head so the SWDGE htil-add latency
            # never bubbles the in-order DVE queue.
            pending = []

            def emit_tail(tail):
                c, ls, LHC, fp, htl, hv = tail
                fpm1 = gates_pool.tile([P, LHC], F16, tag="fpm1")
                nc.vector.tensor_scalar_add(fpm1[:], fp[:], -1.0)
                wv = gates_pool.tile([P, LHC], F16, tag="wv")
                nc.vector.tensor_tensor(wv[:], fpm1[:], htl[:], Alu.mult)
                init = HX_INIT if ls == 0 else hv[:, ls - 1 : ls]
                nc.vector.tensor_tensor_scan(
                    hv[:, ls : ls + LHC], fp[:], wv[:], init,
                    Alu.mult, Alu.subtract,
                )
                nc.sync.dma_start(
                    out_d[c * P : (c + 1) * P, ls : ls + LHC],
                    hv[:, ls : ls + LHC],
                )

            for ei, (c, lh) in enumerate(order):
                if lh == 0:
                    hvs[c] = scan_pool.tile([P, L], F16, tag="hv", name=f"hv{c}")
                hv = hvs[c]
                LHC = clists[c][lh]
                ls = lsoff[c]

                sf = gates_pool.tile([P, LHC], F16, tag="sf")
                si = gates_pool.tile([P, LHC], F16, tag="si")
                fp = tail_pool.tile([P, LHC], F16, tag="fp")
                sh = gates_pool.tile([P, LHC], F16, tag="sh")
                htl = tail_pool.tile([P, LHC], F16, tag="htl")

                def mms8(ps, ocol):
                    # fp8 DoubleRow: 2 k-tiles of 128 contracted per pass
                    for j in range(LHC // LT):
                        xk = slice(ls + j * LT, ls + (j + 1) * LT)
                        jl = slice(j * LT, (j + 1) * LT)
                        for kp in range(NK // 2):
                            nc.tensor.matmul(
                                ps[:, jl],
                                w8_sb[:, 2 * kp : 2 * kp + 2, ocol : ocol + P],
                                xT8_sb[:, 2 * kp : 2 * kp + 2, xk],
                                start=kp == 0,
                                stop=kp == NK // 2 - 1,
                                perf_mode=DR,
                            )

                def mms16(ps, ocol):
                    for j in range(LHC // LT):
                        xk = slice(ls + j * LT, ls + (j + 1) * LT)
                        jl = slice(j * LT, (j + 1) * LT)
                        for ki in range(NK):
                            nc.tensor.matmul(
                                ps[:, jl],
                                w16_sb[:, ki, ocol : ocol + P],
                                xT16_sb[:, ki, xk],
                                start=ki == 0,
                                stop=ki == NK - 1,
                            )

                # F gate
                psF = psum_pool.tile([P, LHC], F32, tag="ps")
                mms8(psF, c * CW8)
                nc.scalar.activation(
                    sf[:], psF[:], Act.Sigmoid,
                    bias=bias_sb[:, 0 * NCT + c : 0 * NCT + c + 1],
                    scale=1.0 / (XS * WS),
                )
                # I gate
                psI = psum_pool.tile([P, LHC], F32, tag="ps")
                mms8(psI, c * CW8 + P)
                nc.scalar.activation(
                    si[:], psI[:], Act.Sigmoid,
                    bias=bias_sb[:, 1 * NCT + c : 1 * NCT + c + 1],
                    scale=1.0 / (XS * WS),
                )
                # f' = sf/(sf+si), one fused DVE op
                nc.vector._custom_dve(
                    fract_op, out=fp[:], in0=sf[:], in1=si[:],
                    s0=FR_C0, s1=FR_C1,
                )

                # H gate
                psH = psum_pool.tile([P, LHC], F32, tag="ps")
                mms16(psH, c * P)
                nc.scalar.activation(
                    sh[:], psH[:], Act.Sigmoid,
                    bias=bias_sb[:, 2 * NCT + c : 2 * NCT + c + 1],
                )
                # htil = relu(psH + b_h) + min(sh, 0.5), exact identity for g:
                # relu on ACT, min on DVE (4x TS), add via SWDGE (off-engine).
                mn = gates_pool.tile([P, LHC], F16, tag="mn")
                nc.scalar.activation(
                    htl[:], psH[:], Act.Relu,
                    bias=bias_sb[:, 2 * NCT + c : 2 * NCT + c + 1],
                )
                nc.vector.tensor_scalar_min(mn[:], sh[:], 0.5)
                nc.gpsimd.dma_start(
                    out=htl[:], in_=mn[:], accum_op=Alu.add
                )
                pending.append((c, ls, LHC, fp, htl, hv))
                if len(pending) > 1:
                    emit_tail(pending.pop(0))
                lsoff[c] += LHC
            while pending:
                emit_tail(pending.pop(0))

    nc.compile()
    _cached_nc[key] = nc
    return nc


def reorder_w8(W: np.ndarray) -> np.ndarray:
    """fp8 weights, [H_IN, NCT*2P] with per-c-tile [F_c | I_c] col groups."""
    Wf, Wi = W[:H], W[H : 2 * H]
    cols = []
    for c in range(NCT):
        cols.append(Wf[c * P : (c + 1) * P])
        cols.append(Wi[c * P : (c + 1) * P])
    w = np.concatenate(cols, axis=0)  # [NCT*2P, H_IN]
    w8 = np.clip(w.T * WS, -240.0, 240.0).astype(ml_dtypes.float8_e4m3fn)
    return np.ascontiguousarray(w8)


def reorder_w16(W: np.ndarray) -> np.ndarray:
    Wh = W[2 * H :]
    return np.ascontiguousarray(Wh.T.astype(np.float16))


def make_bias(b: np.ndarray) -> np.ndarray:
    b32 = np.asarray(b, dtype=np.float32)
    bias = np.empty((P, 16), dtype=np.float32)
    for c in range(NCT):
        bias[:, 0 * NCT + c] = b32[0 * H + c * P : 0 * H + (c + 1) * P]
        bias[:, 1 * NCT + c] = b32[1 * H + c * P : 1 * H + (c + 1) * P]
        bias[:, 2 * NCT + c] = b32[2 * H + c * P : 2 * H + (c + 1) * P]
        bias[:, 3 * NCT + c] = b32[2 * H + c * P : 2 * H + (c + 1) * P] + 0.5
    return bias


def _prep_core_inputs(x_n, w16, w8, bias):
    xT = np.ascontiguousarray(x_n.T)
    return {
        "xT16": xT.astype(np.float16),
        "xT8": np.clip(xT * XS, -240.0, 240.0).astype(ml_dtypes.float8_e4m3fn),
        "w16": w16,
        "w8": w8,
        "bias": bias,
    }


def kernel(x: np.ndarray, W: np.ndarray, b: np.ndarray) -> np.ndarray:
    from concourse.bass_utils import run_bass_kernel_spmd

    nc = build_program()

    W = np.asarray(W)
    w16 = reorder_w16(W)
    w8 = reorder_w8(W)
    bias = make_bias(b)

    in_maps = [_prep_core_inputs(np.asarray(x[n]), w16, w8, bias) for n in range(N)]
    res = run_bass_kernel_spmd(nc, in_maps, list(range(N)))

    out = np.empty((N, L, H), dtype=np.float32)
    for n in range(N):
        out[n] = res.results[n]["out"].T.astype(np.float32)
    return out


# revision 26
# speedup vs baseline: 1.0546x; 1.0033x over previous
"""MinLSTM cell (Heinsen-scan reference) as a Bass/Tile kernel for 8 trn2 NeuronCores.

Linear-space rewrite of the reference's log-space scan:
    h_t = f'_t h_{t-1} + (1 - f'_t) g(pre_h_t),   h_0 = 1e-6
with f' = sigmoid(pre_f+b_f) / (sigmoid(pre_f+b_f) + sigmoid(pre_i+b_i)) and
g(x) = x>=0 ? x+0.5 : sigmoid(x) = max(x+0.5, sigmoid(x)).

Distribution: data-parallel over batch N=8, one batch element per core, W/b
replicated. Device layout: channels on SBUF partitions (4 c-tiles of 128),
sequence along the free dim.

Per-core engine assignment (chunks of <=2048 along L):
  PE : F/I gate matmuls in fp8 E4M3 (x*16, W*64) with DoubleRow perf mode
       (2 k-tiles of 128 contracted per pass), H gate matmul in fp16.
  ACT: sf = sigmoid(psF/1024 + b_f); si = sigmoid(psI/1024 + b_i);
       sh = sigmoid(psH + b_h); on ACT-chunks also rl = psH + b_h + 0.5.
  DVE: fp = FRACT_FAST_ANT(sf, si) = sf/(sf+si), a custom fused op:
       bit-trick reciprocal seed + one Newton step + multiply (~0.17% max
       rel err), replacing the SWDGE add + fp32 recip + multiply chain.
       On DVE-chunks htil = HTIL_FUSED_ANT(psH, sh) = max(psH+b_h05, sh).
       h = tensor_tensor_scan(d0=fp, d1=wv, mult, subtract).
  GPS: fpm1 = fp - 1 (tensor_scalar); SWDGE accumulate-max (sh into rl ->
       htil, ACT-chunks) and accumulate-mult (htil into fpm1 -> wv).
  SP : all HBM loads/stores.
"""

import os
import sys

import numpy as np

sys.path.insert(0, "/opt/trn_rl_repo")

import ml_dtypes  # noqa: E402

import concourse.bass as bass  # noqa: E402
import concourse.tile as tile  # noqa: E402
from concourse import bacc, mybir  # noqa: E402
from concourse import dve_ops  # noqa: E402
from concourse.dve_spec import (  # noqa: E402
    AluOp,
    Bin,
    C0,
    C1,
    Spec,
    lower,
    maxx,
)
from concourse.dve_uop import DveOpSpec  # noqa: E402

N, L, H_IN, H = 8, 4096, 512, 512
H3 = 3 * H
P = 128
NK = H_IN // P  # 4 k-blocks of the contraction dim
NCT = H // P  # 4 channel tiles
LT = 512  # psum/matmul l-tile (one PSUM bank of fp32)
LH = 2048  # l-granularity of the big elementwise ops
F32 = mybir.dt.float32
F16 = mybir.dt.float16
F8 = mybir.dt.float8e4
Alu = mybir.AluOpType
Act = mybir.ActivationFunctionType
DR = mybir.MatmulPerfMode.DoubleRow

HX_INIT = 1e-6
XS, WS = 16.0, 64.0  # fp8 scale for x and W (TRN E4M3 max is +-240)
FR_C0, FR_C1 = -0.23549792, 2.0017324  # recip bit-seed Chebyshev consts

_cached_nc = {}
_fract_op = None
_htil_op = None


def _register_dve_ops():
    """Author + register the two fused custom DVE ops (process-local registry,
    compiled into the NEFF's per-kernel DVE table)."""
    global _fract_op, _htil_op
    if _fract_op is not None:
        return _fract_op, _htil_op

    def _np_recip_seed_nr1(s, c0, c1):
        ns = (~s.view(np.int32)).view(np.float32)
        y0 = ns * c0
        return y0 * (c1 - s * y0)

    def _ref_fract(in0, in1, c0, c1, c2):
        sf = in0.astype(np.float32)
        s = sf + in1.astype(np.float32)
        return sf * _np_recip_seed_nr1(s, c0, c1)

    _s = Src0 = None  # placeholder to appease linters
    from concourse.dve_spec import Src0, Src1  # noqa: E402

    s_expr = Src0 + Src1
    not_s = Bin(AluOp.BITWISE_NOT, s_expr, s_expr)
    y0 = not_s * C0
    y1 = y0 * (C1 - s_expr * y0)
    fract_spec = Spec(body=Src0 * y1, reference=_ref_fract)

    def _ref_htil(in0, in1, c0, c1, c2):
        return np.maximum(in0.astype(np.float32) + c0, in1.astype(np.float32))

    htil_spec = Spec(body=maxx(Src0 + C0, Src1), reference=_ref_htil)

    ops = []
    for name, spec in (
        ("FRACT_FAST_ANT", fract_spec),
        ("HTIL_FUSED_ANT", htil_spec),
    ):
        existing = next((o for o in dve_ops.OPS if o.name == name), None)
        if existing is not None:
            ops.append(existing)
            continue
        row = dve_ops._CUSTOM_DVE_ROW_BASE + len(dve_ops.OPS)
        shas = {}
        for ver in ("v3",):
            tmp = DveOpSpec(
                name=name,
                opcode=row,
                uops=lower(spec, ver=ver),
                rd1_en=True,
            )
            shas[ver] = tmp.sha(ver)
        op = dve_ops.DveOp(name=name, spec=spec, subdim=False, uops_sha=shas)
        dve_ops.OPS.append(op)
        dve_ops._SUB_OPCODE_FOR_NAME[name] = row
        dve_ops.CUSTOM_DVE_SPECS[name] = spec
        ops.append(op)
    _fract_op, _htil_op = ops
    return _fract_op, _htil_op


def build_program(L=L, LH=LH):
    key = (L, LH)
    if key in _cached_nc:
        return _cached_nc[key]
    fract_op, htil_op = _register_dve_ops()

    nc = bacc.Bacc()
    xT16_d = nc.dram_tensor("xT16", [H_IN, L], F16, kind="ExternalInput")
    xT8_d = nc.dram_tensor("xT8", [H_IN, L], F8, kind="ExternalInput")
    w16_d = nc.dram_tensor("w16", [H_IN, NCT * P], F16, kind="ExternalInput")
    w8_d = nc.dram_tensor("w8", [H_IN, NCT * 2 * P], F8, kind="ExternalInput")
    bias_d = nc.dram_tensor("bias", [P, 16], F32, kind="ExternalInput")
    out_d = nc.dram_tensor("out", [H, L], F16, kind="ExternalOutput")

    CW8 = 2 * P  # fp8 weight cols per c-tile: [F_c | I_c]

    with tile.TileContext(nc) as tc:
        with (
            tc.tile_pool(name="const", bufs=1) as const_pool,
            tc.tile_pool(name="gates", bufs=2) as gates_pool,
            tc.tile_pool(name="tail", bufs=3) as tail_pool,
            tc.tile_pool(name="scanbuf", bufs=2) as scan_pool,
            tc.tile_pool(name="psum", bufs=2, space="PSUM") as psum_pool,
        ):
            # Warmup activation: absorbs the one-time sigmoid act-table load.
            warm = const_pool.tile([P, 8], F32)
            nc.vector.memset(warm[:], 0.0)
            nc.scalar.activation(warm[:], warm[:], Act.Sigmoid)
            # PE warmup: garbage matmuls with no deps so the HAM clock gate
            # reaches 2.4GHz while the first DMAs are in flight.
            wup = const_pool.tile([P, P], F16)
            nc.vector.memset(wup[:], 0.0)
            wup_ps = psum_pool.tile([P, P], F32, tag="ps")
            for _ in range(16):
                nc.tensor.matmul(wup_ps[:], wup[:], wup[:], start=True, stop=True)

            xT16_sb = const_pool.tile([P, NK, L], F16)
            xT8_sb = const_pool.tile([P, NK, L], F8)
            w16_sb = const_pool.tile([P, NK, NCT * P], F16)
            w8_sb = const_pool.tile([P, NK, NCT * CW8], F8)
            bias_sb = const_pool.tile([P, 16], F32)

            # Load order prioritizes chunk-0's critical path (warmup is short,
            # so the first real matmuls need w8-c0 + x8[0:512] ASAP), then
            # streams the remaining weights and x chunks.
            w8_r = w8_d.rearrange("(ki p) o -> p ki o", p=P)
            w16_r = w16_d.rearrange("(ki p) o -> p ki o", p=P)
            xT16_r = xT16_d.rearrange("(ki p) l -> p ki l", p=P)
            xT8_r = xT8_d.rearrange("(ki p) l -> p ki l", p=P)
            if L >= 4096:
                xchunks = [512, 512, 1024] + [2048] * ((L - 2048) // 2048)
            else:
                xchunks = [512] * (L // 512)

            nc.sync.dma_start(w8_sb[:, :, 0:CW8], w8_r[:, :, 0:CW8])
            nc.sync.dma_start(
                xT8_sb[:, :, 0 : xchunks[0]], xT8_r[:, :, 0 : xchunks[0]]
            )
            nc.sync.dma_start(w16_sb[:, :, 0:P], w16_r[:, :, 0:P])
            nc.sync.dma_start(bias_sb[:], bias_d[:])
            nc.sync.dma_start(
                xT16_sb[:, :, 0 : xchunks[0]], xT16_r[:, :, 0 : xchunks[0]]
            )
            for cg in range(1, NCT):
                nc.sync.dma_start(
                    w8_sb[:, :, cg * CW8 : (cg + 1) * CW8],
                    w8_r[:, :, cg * CW8 : (cg + 1) * CW8],
                )
                nc.sync.dma_start(
                    w16_sb[:, :, cg * P : (cg + 1) * P],
                    w16_r[:, :, cg * P : (cg + 1) * P],
                )
            xoff = xchunks[0]
            for xch in xchunks[1:]:
                nc.sync.dma_start(
                    xT8_sb[:, :, xoff : xoff + xch],
                    xT8_r[:, :, xoff : xoff + xch],
                )
                nc.sync.dma_start(
                    xT16_sb[:, :, xoff : xoff + xch],
                    xT16_r[:, :, xoff : xoff + xch],
                )
                xoff += xch

            # Near-c-major emission with one swap: c1's small first chunk is
            # emitted before c0's last chunk (hides the c0->c1 seam on PE).
            if L >= 4096:
                big = (L - 4096) // 2048
                clists = {
                    0: [512, 512, 1024] + [2048] * (big + 1),
                    1: [512, 1536] + [2048] * (big + 1),
                    2: [2048] * (big + 2),
                    3: [2048] * (big + 1) + [1536, 512],
                }
                # Near-c-major with one hoist (c1's small first chunk before
                # c0's last): hides the c0->c1 seam. c3 trails small chunks so
                # the exposed end-of-kernel tail chain is short.
                order = [(0, 0), (0, 1), (0, 2), (1, 0), (0, 3)]
                order += [(1, j) for j in range(1, len(clists[1]))]
                order += [(2, j) for j in range(len(clists[2]))]
                order += [(3, j) for j in range(len(clists[3]))]
            else:
                clists = {c: [512] * (L // 512) for c in range(NCT)}
                order = [(c, j) for c in range(NCT)
                         for j in range(len(clists[c]))]

            hvs = {}
            lsoff = {c: 0 for c in range(NCT)}
            # Software pipelining: each chunk's (fpm1, wv, scan, store) tail is
            # emitted AFTER the next chunk's head so the SWDGE htil-add latency
            # never bubbles the in-order DVE queue.
            pending = []

            def emit_tail(tail):
                c, ls, LHC, fp, htl, hv = tail
                fpm1 = gates_pool.tile([P, LHC], F16, tag="fpm1")
                nc.vector.tensor_scalar_add(fpm1[:], fp[:], -1.0)
                wv = gates_pool.tile([P, LHC], F16, tag="wv")
                nc.vector.tensor_tensor(wv[:], fpm1[:], htl[:], Alu.mult)
                init = HX_INIT if ls == 0 else hv[:, ls - 1 : ls]
                nc.vector.tensor_tensor_scan(
                    hv[:, ls : ls + LHC], fp[:], wv[:], init,
                    Alu.mult, Alu.subtract,
                )
                nc.sync.dma_start(
                    out_d[c * P : (c + 1) * P, ls : ls + LHC],
                    hv[:, ls : ls + LHC],
                )

            for ei, (c, lh) in enumerate(order):
                if lh == 0:
                    hvs[c] = scan_pool.tile([P, L], F16, tag="hv", name=f"hv{c}")
                hv = hvs[c]
                LHC = clists[c][lh]
                ls = lsoff[c]

                sf = gates_pool.tile([P, LHC], F16, tag="sf")
                si = gates_pool.tile([P, LHC], F16, tag="si")
                fp = tail_pool.tile([P, LHC], F16, tag="fp")
                sh = gates_pool.tile([P, LHC], F16, tag="sh")
                htl = tail_pool.tile([P, LHC], F16, tag="htl")

                def mms8(ps, ocol):
                    # fp8 DoubleRow: 2 k-tiles of 128 contracted per pass
                    for j in range(LHC // LT):
                        xk = slice(ls + j * LT, ls + (j + 1) * LT)
                        jl = slice(j * LT, (j + 1) * LT)
                        for kp in range(NK // 2):
                            nc.tensor.matmul(
                                ps[:, jl],
                                w8_sb[:, 2 * kp : 2 * kp + 2, ocol : ocol + P],
                                xT8_sb[:, 2 * kp : 2 * kp + 2, xk],
                                start=kp == 0,
                                stop=kp == NK // 2 - 1,
                                perf_mode=DR,
                            )

                def mms16(ps, ocol):
                    for j in range(LHC // LT):
                        xk = slice(ls + j * LT, ls + (j + 1) * LT)
                        jl = slice(j * LT, (j + 1) * LT)
                        for ki in range(NK):
                            nc.tensor.matmul(
                                ps[:, jl],
                                w16_sb[:, ki, ocol : ocol + P],
                                xT16_sb[:, ki, xk],
                                start=ki == 0,
                                stop=ki == NK - 1,
                            )

                # F gate
                psF = psum_pool.tile([P, LHC], F32, tag="ps")
                mms8(psF, c * CW8)
                nc.scalar.activation(
                    sf[:], psF[:], Act.Sigmoid,
                    bias=bias_sb[:, 0 * NCT + c : 0 * NCT + c + 1],
                    scale=1.0 / (XS * WS),
                )
                # I gate
                psI = psum_pool.tile([P, LHC], F32, tag="ps")
                mms8(psI, c * CW8 + P)
                nc.scalar.activation(
                    si[:], psI[:], Act.Sigmoid,
                    bias=bias_sb[:, 1 * NCT + c : 1 * NCT + c + 1],
                    scale=1.0 / (XS * WS),
                )
                # f' = sf/(sf+si), one fused DVE op
                nc.vector._custom_dve(
                    fract_op, out=fp[:], in0=sf[:], in1=si[:],
                    s0=FR_C0, s1=FR_C1,
                )

                # H gate
                psH = psum_pool.tile([P, LHC], F32, tag="ps")
                mms16(psH, c * P)
                nc.scalar.activation(
                    sh[:], psH[:], Act.Sigmoid,
                    bias=bias_sb[:, 2 * NCT + c : 2 * NCT + c + 1],
                )
                # htil = relu(psH + b_h) + min(sh, 0.5), exact identity for g:
                # relu on ACT, min on DVE (4x TS), add via SWDGE (off-engine).
                mn = gates_pool.tile([P, LHC], F16, tag="mn")
                nc.scalar.activation(
                    htl[:], psH[:], Act.Relu,
                    bias=bias_sb[:, 2 * NCT + c : 2 * NCT + c + 1],
                )
                nc.vector.tensor_scalar_min(mn[:], sh[:], 0.5)
                nc.gpsimd.dma_start(
                    out=htl[:], in_=mn[:], accum_op=Alu.add
                )
                pending.append((c, ls, LHC, fp, htl, hv))
                # Deferral hides SWDGE latency in the throughput-bound steady
                # state, but in the latency-bound first chunks it makes wv
                # queue behind the NEXT chunk's ACT-gated ops on the in-order
                # DVE; emit those tails immediately instead.
                depth = 0 if ei < 3 else 1
                while len(pending) > depth:
                    emit_tail(pending.pop(0))
                lsoff[c] += LHC
            while pending:
                emit_tail(pending.pop(0))

    nc.compile()
    _cached_nc[key] = nc
    return nc


def reorder_w8(W: np.ndarray) -> np.ndarray:
    """fp8 weights, [H_IN, NCT*2P] with per-c-tile [F_c | I_c] col groups."""
    Wf, Wi = W[:H], W[H : 2 * H]
    cols = []
    for c in range(NCT):
        cols.append(Wf[c * P : (c + 1) * P])
        cols.append(Wi[c * P : (c + 1) * P])
    w = np.concatenate(cols, axis=0)  # [NCT*2P, H_IN]
    w8 = np.clip(w.T * WS, -240.0, 240.0).astype(ml_dtypes.float8_e4m3fn)
    return np.ascontiguousarray(w8)


def reorder_w16(W: np.ndarray) -> np.ndarray:
    Wh = W[2 * H :]
    return np.ascontiguousarray(Wh.T.astype(np.float16))


def make_bias(b: np.ndarray) -> np.ndarray:
    b32 = np.asarray(b, dtype=np.float32)
    bias = np.empty((P, 16), dtype=np.float32)
    for c in range(NCT):
        bias[:, 0 * NCT + c] = b32[0 * H + c * P : 0 * H + (c + 1) * P]
        bias[:, 1 * NCT + c] = b32[1 * H + c * P : 1 * H + (c + 1) * P]
        bias[:, 2 * NCT + c] = b32[2 * H + c * P : 2 * H + (c + 1) * P]
        bias[:, 3 * NCT + c] = b32[2 * H + c * P : 2 * H + (c + 1) * P] + 0.5
    return bias


def _prep_core_inputs(x_n, w16, w8, bias):
    xT = np.ascontiguousarray(x_n.T)
    return {
        "xT16": xT.astype(np.float16),
        "xT8": np.clip(xT * XS, -240.0, 240.0).astype(ml_dtypes.float8_e4m3fn),
        "w16": w16,
        "w8": w8,
        "bias": bias,
    }


def kernel(x: np.ndarray, W: np.ndarray, b: np.ndarray) -> np.ndarray:
    from concourse.bass_utils import run_bass_kernel_spmd

    nc = build_program()

    W = np.asarray(W)
    w16 = reorder_w16(W)
    w8 = reorder_w8(W)
    bias = make_bias(b)

    in_maps = [_prep_core_inputs(np.asarray(x[n]), w16, w8, bias) for n in range(N)]
    res = run_bass_kernel_spmd(nc, in_maps, list(range(N)))

    out = np.empty((N, L, H), dtype=np.float32)
    for n in range(N):
        out[n] = res.results[n]["out"].T.astype(np.float32)
    return out
